# revision 1
# baseline (speedup 1.0000x reference)
"""AdaAggLayer Trainium2 kernel — 1D Winograd F(2,3) along W.

Data-parallel over batch: 8 NeuronCores x 4 samples each.

The 3x3 conv is decomposed as Winograd F(2,3) along the width axis only:
per (kh row, output-column pair) the 3 kw taps become 4 Winograd taps, so
the PE does 12 tap-matmuls per output instead of 18 shifted matmuls — a
1.5x cut in TensorE cycles (the roofline engine). The tap transform of the
weights rides the host-side align fold (both are weight-only, exact f32);
aggregation then happens directly in tap space on DVE. The input taps
  t0 = xe[j]-xe[j+1], t1 = xo[j]+xe[j+1], t2 = xe[j+1]-xo[j],
  t3 = xo[j]-xo[j+1]
are built from host-split even/odd column planes (pure layout) as
contiguous tensor_tensor adds on DVE (never GpSimd: concurrent Pool
tensor ops degrade DVE ~6x via SBUF contention). The inverse transform
  y_even = e0+e1+e2, y_odd = e1-e2-e3
runs on DVE in bf16 straight off the ACT evacuations (bias rides e1's
evac since its coefficient is +1 in both outputs). Output is stored as
[h, parity, w'] and interleaved on the host (pure layout).
"""

import contextlib
import importlib.util
import sys
import types

sys.path.insert(0, "/opt/trn_rl_repo")

import numpy as np
import ml_dtypes

import concourse.bass as bass
import concourse.mybir as mybir
import concourse.tile as tile
from concourse import bacc
from concourse.bass_utils import run_bass_kernel_spmd

N_CORES = 8
B, I, O, E, HID = 32, 256, 256, 5, 65
H = W = 56
HP = H + 2  # zero-padded spatial rows
WE = 29  # even/odd column plane width (padded 58 cols split)
WT = 28  # winograd output-pair columns
KH = 3
TAP = 4
NBLK = 4  # row blocks of 14 output rows
RB = 14
BF16 = mybir.dt.bfloat16
F32 = mybir.dt.float32

_NC_CACHE = None


def _install_ntff_hook():
    """Register the axon NTFF profiling hook (the image's antenv lacks it)."""
    if "antenv.axon_hooks" in sys.modules:
        return
    try:
        spec = importlib.util.spec_from_file_location(
            "trn_boot", "/root/.axon_site/trn_agent_boot/trn_boot.py"
        )
        tb = importlib.util.module_from_spec(spec)
        spec.loader.exec_module(tb)
        hook = tb._ntff_profile_via_ctypes("/opt/axon/libaxon_pjrt.so")
    except Exception:
        hook = None
    mod = types.ModuleType("antenv.axon_hooks")
    mod.get_axon_ntff_profile_hook = lambda: hook
    sys.modules["antenv.axon_hooks"] = mod


def _emit(nc, tc, ctx):
    xe_d = nc.dram_tensor("xe", [4, I, HP, WE], BF16, kind="ExternalInput")
    xo_d = nc.dram_tensor("xo", [4, I, HP, WE], BF16, kind="ExternalInput")
    # tap-transformed aligned weights, chunk-ordered:
    # [kh, ot, p=i%128, e, tap, ih, o128]
    w_d = nc.dram_tensor("wt", [KH, 2, 128, E, TAP, 2, 128], BF16, kind="ExternalInput")
    # packed small constants: [p, 0:130]=w1T (2 ih), [0:66, 130:135]=w2Ta,
    # [0:5, 135:391]=bias. One DMA: each dma_start costs ~650ns of serial
    # trigger issue on the sync sequencer.
    misc_d = nc.dram_tensor("misc", [128, 391], F32, kind="ExternalInput")
    out_d = nc.dram_tensor("out", [4, O, H, 2, WT], BF16, kind="ExternalOutput")

    const = ctx.enter_context(tc.tile_pool(name="const", bufs=1))
    xpl = ctx.enter_context(tc.tile_pool(name="xpl", bufs=1))
    xtp = ctx.enter_context(tc.tile_pool(name="xt", bufs=1))
    aggp = ctx.enter_context(tc.tile_pool(name="agg", bufs=1))
    tmpp = ctx.enter_context(tc.tile_pool(name="tmp", bufs=2))
    evp = ctx.enter_context(tc.tile_pool(name="ev", bufs=1))
    ytp = ctx.enter_context(tc.tile_pool(name="yt", bufs=3))
    stagep = ctx.enter_context(tc.tile_pool(name="stage", bufs=4))
    s_psum = ctx.enter_context(tc.tile_pool(name="sps", bufs=1, space="PSUM"))
    t_psum = ctx.enter_context(tc.tile_pool(name="tps", bufs=7, space="PSUM"))

    # ---- SBUF constants ----
    walTt = const.tile([128, KH, 2, E, TAP, 2, 128], BF16)
    pscr = const.tile([128, 2, 29, WE], BF16)  # pooling tree scratch
    misc_sb = const.tile([128, 391], F32)
    w1_v = lambda ih: misc_sb[:, ih * HID : (ih + 1) * HID]
    w2_sb = misc_sb[: HID + 1, 130:135]
    bias_sb = misc_sb[:E, 135:391]
    ones_sb = const.tile([1, 128], F32)
    pooledT = const.tile([128, 2, 4], F32)
    h_sb = const.tile([HID + 1, 4], F32)  # row HID is constant 1.0
    att_sb = const.tile([E, 4], F32)
    att_row = const.tile([1, 4 * E], F32)
    att_bc = const.tile([128, 4, E], F32)
    aggb_sb = const.tile([128, 2, 4], F32)

    xe_sb = {}
    xt_sb = {}

    def dma_xplanes(b):
        for ih in range(2):
            te = xpl.tile([128, HP, WE], BF16, tag=f"xe_{ih}", bufs=2, name=f"xe{b}_{ih}")
            to = xpl.tile([128, HP, WE], BF16, tag=f"xo_{ih}", bufs=2, name=f"xo{b}_{ih}")
            nc.sync.dma_start(out=te[:, :, :], in_=xe_d[b, ih * 128 : (ih + 1) * 128])
            nc.sync.dma_start(out=to[:, :, :], in_=xo_d[b, ih * 128 : (ih + 1) * 128])
            xe_sb[(b, ih)] = (te, to)

    def build_xt_taps(b, ih, taps):
        # winograd input taps as contiguous tensor_tensor ops (DVE 2x).
        # NOTE: never place these on gpsimd — concurrent Pool tensor ops
        # degrade DVE throughput ~6x (SBUF contention).
        if (b, ih) in xt_sb:
            t = xt_sb[(b, ih)]
        else:
            t = xtp.tile(
                [128, TAP, HP, WT], BF16, tag=f"xt_{ih}", bufs=2, name=f"xt{b}_{ih}"
            )
            xt_sb[(b, ih)] = t
        xe, xo = xe_sb[(b, ih)]
        for tap in taps:
            if tap == 0:
                nc.vector.tensor_sub(out=t[:, 0], in0=xe[:, :, 0:28], in1=xe[:, :, 1:29])
            elif tap == 1:
                nc.vector.tensor_add(out=t[:, 1], in0=xo[:, :, 0:28], in1=xe[:, :, 1:29])
            elif tap == 2:
                nc.vector.tensor_sub(out=t[:, 2], in0=xe[:, :, 1:29], in1=xo[:, :, 0:28])
            else:
                nc.vector.tensor_sub(out=t[:, 3], in0=xo[:, :, 0:28], in1=xo[:, :, 1:29])

    def _pool_tree(b, ih, ncols):
        s = pscr[:, ih]
        nc.vector.tensor_add(
            out=s[:, 0:14, 0:ncols], in0=s[:, 0:14, 0:ncols], in1=s[:, 15:29, 0:ncols]
        )
        nc.vector.tensor_add(
            out=s[:, 0:7, 0:ncols], in0=s[:, 0:7, 0:ncols], in1=s[:, 7:14, 0:ncols]
        )
        nc.vector.tensor_add(
            out=s[:, 0:1, 0:ncols], in0=s[:, 0:1, 0:ncols], in1=s[:, 14:15, 0:ncols]
        )
        nc.vector.reduce_sum(
            out=pooledT[:, ih, b : b + 1],
            in_=s[:, 0:7, 0:ncols],
            axis=mybir.AxisListType.XY,
        )

    def pool_half_raw(b, ih):
        # pooling for samples whose x_t is not built yet (avoids blocking the
        # DVE queue on the x_t tile ring): fold xe+xo then the row tree.
        s = pscr[:, ih]
        xe, xo = xe_sb[(b, ih)]
        nc.vector.tensor_add(out=s[:, 0:29, :], in0=xe[:, 0:29, :], in1=xe[:, 29:58, :])
        nc.vector.tensor_add(out=s[:, 0:29, :], in0=s[:, 0:29, :], in1=xo[:, 0:29, :])
        nc.vector.tensor_add(out=s[:, 0:29, :], in0=s[:, 0:29, :], in1=xo[:, 29:58, :])
        _pool_tree(b, ih, WE)

    def pool_half(b, ih):
        # pooled sum from winograd tap 1: sum_j (xo[j]+xe[j+1]) telescopes to
        # the full (zero-padded) row sum, so the tap plane doubles as the
        # pooling input. Pairwise row-fold tree in 2x bf16.
        s = pscr[:, ih]
        t1 = xt_sb[(b, ih)][:, 1]
        nc.vector.tensor_add(
            out=s[:, 0:29, 0:WT], in0=t1[:, 0:29, :], in1=t1[:, 29:58, :]
        )
        _pool_tree(b, ih, WT)

    def attention_tail(b0, nb, hp):
        nc.scalar.activation(
            h_sb[:HID, b0 : b0 + nb], hp[:, :nb], mybir.ActivationFunctionType.Relu
        )
        ap = s_psum.tile([E, 4], F32, tag="sps", name="ap")
        nc.tensor.matmul(ap[:, :nb], lhsT=w2_sb[:, :], rhs=h_sb[:, b0 : b0 + nb])
        nc.scalar.activation(
            att_sb[:, b0 : b0 + nb], ap[:, :nb], mybir.ActivationFunctionType.Sigmoid
        )
        rp = s_psum.tile([1, 4 * E], F32, tag="sps", name="rp")
        for j in range(nb):
            nc.tensor.matmul(
                rp[0:1, j * E : (j + 1) * E],
                lhsT=h_sb[:, b0 + j : b0 + j + 1],
                rhs=w2_sb[:, :],
            )
        nc.scalar.activation(
            att_row[0:1, b0 * E : (b0 + nb) * E],
            rp[0:1, : nb * E],
            mybir.ActivationFunctionType.Sigmoid,
        )
        bp = s_psum.tile([128, 4 * E], F32, tag="sps", name="bp")
        nc.tensor.matmul(
            bp[:, : nb * E],
            lhsT=ones_sb[0:1, :],
            rhs=att_row[0:1, b0 * E : (b0 + nb) * E],
        )
        nc.vector.tensor_copy(out=att_bc[:, b0 : b0 + nb, :], in_=bp[:, : nb * E])
        for ot in range(2):
            gp = s_psum.tile([128, 4], F32, tag="sps", name="gp")
            nc.tensor.matmul(
                gp[:, :nb],
                lhsT=bias_sb[:, ot * 128 : (ot + 1) * 128],
                rhs=att_sb[:, b0 : b0 + nb],
            )
            nc.vector.tensor_copy(out=aggb_sb[:, ot, b0 : b0 + nb], in_=gp[:, :nb])

    def attention_n(b0, nb):
        hp = s_psum.tile([HID, 4], F32, tag="sps", name="hp")
        for ih in range(2):
            nc.tensor.matmul(
                hp[:, :nb],
                lhsT=w1_v(ih),
                rhs=pooledT[:, ih, b0 : b0 + nb],
                start=(ih == 0),
                stop=(ih == 1),
            )
        attention_tail(b0, nb, hp)

    # per-sample tap-space aggregation on DVE: tensor_scalar 4x muls + 2x adds
    aggs_all = {}

    def agg_chunk(b, kh, ot, split=False):
        # experts 1,2 scaled on ACT (activation Identity with per-partition
        # scale) to shed DVE work; DVE does the other muls (4x) + adds (2x).
        agg = aggp.tile(
            [128, TAP, 2, 128], BF16, tag=f"agg_{kh}_{ot}", bufs=2, name=f"agg{b}_{kh}_{ot}"
        )
        parts = [(0, 2), (2, 4)] if split else [(0, 4)]
        for q0, q1 in parts:
            acts = {}
            for e in (1, 2):
                ta = tmpp.tile([128, TAP, 2, 128], BF16, tag=f"tmpa{e}", name="ta")
                nc.scalar.activation(
                    ta[:, q0:q1],
                    walTt[:, kh, ot, e, q0:q1],
                    mybir.ActivationFunctionType.Identity,
                    scale=att_bc[:, b, e : e + 1],
                )
                acts[e] = ta
            nc.vector.tensor_scalar_mul(
                agg[:, q0:q1], walTt[:, kh, ot, 0, q0:q1], att_bc[:, b, 0:1]
            )
            for e in (3, 4):
                tmp = tmpp.tile(
                    [128, TAP, 2, 128], BF16, tag=f"tmp{e}", bufs=1, name="tmp"
                )
                nc.vector.tensor_scalar_mul(
                    tmp[:, q0:q1], walTt[:, kh, ot, e, q0:q1], att_bc[:, b, e : e + 1]
                )
                acts[e] = tmp
            for e in (1, 2, 3, 4):
                nc.vector.tensor_add(
                    out=agg[:, q0:q1], in0=agg[:, q0:q1], in1=acts[e][:, q0:q1]
                )
        aggs_all[(b, kh, ot)] = agg

    # ---- DMA schedule ----
    dma_xplanes(0)
    nc.sync.dma_start(out=misc_sb[:, :], in_=misc_d[:, :])
    nc.vector.memset(ones_sb[:, :], 1.0)
    nc.vector.memset(h_sb[HID - 1 : HID + 1, :], 1.0)
    for ot in range(2):
        for kh in range(KH):
            nc.sync.dma_start(out=walTt[:, kh, ot], in_=w_d[kh, ot])
    dma_xplanes(1)

    # preload the sigmoid ACT table off the critical path
    tscr = const.tile([1, 4], F32, name="tscr")
    nc.scalar.activation(
        tscr[0:1, :], ones_sb[0:1, 0:4], mybir.ActivationFunctionType.Sigmoid
    )

    for ih in range(2):
        build_xt_taps(0, ih, [1])
    for ih in range(2):
        pool_half(0, ih)
    attention_n(0, 1)
    for ih in range(2):
        build_xt_taps(0, ih, [0, 2, 3])
    agg_chunk(0, 0, 0, split=True)
    agg_chunk(0, 1, 0)
    agg_chunk(0, 2, 0)
    for ih in range(2):
        build_xt_taps(1, ih, [1])
    for ih in range(2):
        pool_half(1, ih)
    for kh in range(KH):
        agg_chunk(0, kh, 1)

    # ---- per-sample winograd conv ----
    for b in range(4):
        if b >= 1:
            for ot in range(2):
                for kh in range(KH):
                    agg_chunk(b, kh, ot)
        for ot in range(2):
            for pair in range(2):
                ev = {
                    tap: evp.tile(
                        [128, 2, RB, WT], BF16, tag=f"e{tap}", bufs=2, name=f"e{tap}"
                    )
                    for tap in range(TAP)
                }
                for sub in range(2):
                    blk = pair * 2 + sub
                    r0 = blk * RB
                    tp = {
                        tap: t_psum.tile([128, RB, WT], F32, tag="tap", name=f"tp{tap}")
                        for tap in range(TAP)
                    }
                    for kh in range(KH):
                        agg = aggs_all[(b, kh, ot)]
                        for tap in range(TAP):
                            for ih in range(2):
                                nc.tensor.matmul(
                                    tp[tap][:, :, :],
                                    lhsT=agg[:, tap, ih, :],
                                    rhs=xt_sb[(b, ih)][
                                        :, tap, r0 + kh : r0 + kh + RB, :
                                    ],
                                    start=(kh == 0 and ih == 0),
                                    stop=(kh == KH - 1 and ih == 1),
                                )
                    # evac taps to bf16; bias rides e1 (coefficient +1 in
                    # both winograd outputs)
                    for tap in range(TAP):
                        if tap == 1:
                            nc.scalar.activation(
                                ev[tap][:, sub],
                                tp[tap][:, :, :],
                                mybir.ActivationFunctionType.Identity,
                                bias=aggb_sb[:, ot, b : b + 1],
                            )
                        else:
                            nc.scalar.activation(
                                ev[tap][:, sub],
                                tp[tap][:, :, :],
                                mybir.ActivationFunctionType.Identity,
                            )
                # inverse transform on DVE (bf16 2x), both blocks at once,
                # into the parity-split stage
                st = stagep.tile([128, 2, RB, 2, WT], BF16, tag="stage", bufs=3, name="st")
                y0t = ytp.tile([128, 2, RB, WT], BF16, tag="yt", name="y0t")
                nc.vector.tensor_add(out=y0t, in0=ev[0][:, :, :, :], in1=ev[1][:, :, :, :])
                nc.vector.tensor_add(out=st[:, :, :, 0, :], in0=y0t[:, :, :, :], in1=ev[2][:, :, :, :])
                y1t = ytp.tile([128, 2, RB, WT], BF16, tag="yt", name="y1t")
                nc.vector.tensor_sub(out=y1t, in0=ev[1][:, :, :, :], in1=ev[2][:, :, :, :])
                nc.vector.tensor_sub(out=st[:, :, :, 1, :], in0=y1t[:, :, :, :], in1=ev[3][:, :, :, :])
                nc.sync.dma_start(
                    out=out_d[b, ot * 128 : (ot + 1) * 128, pair * 2 * RB : (pair + 1) * 2 * RB, :, :],
                    in_=st[:, :, :, :, :],
                )
            # pipeline hooks: next sample's attention + remaining input taps
            # after ot0; the sample after that gets its tap-1 plane + pooling
            # after ot1 (its x lands mid-conv).
            if ot == 0 and b < 3:
                attention_n(b + 1, 1)
                for ih in range(2):
                    build_xt_taps(b + 1, ih, [0, 2, 3])
                if b + 2 < 4:
                    dma_xplanes(b + 2)
            if ot == 1 and b < 2:
                for ih in range(2):
                    build_xt_taps(b + 2, ih, [1])
                for ih in range(2):
                    pool_half(b + 2, ih)


def _build():
    nc = bacc.Bacc("TRN2", target_bir_lowering=False, debug=False, num_devices=N_CORES)
    with contextlib.ExitStack() as ctx:
        tc = ctx.enter_context(tile.TileContext(nc))
        _emit(nc, tc, ctx)
    nc.compile()
    return nc


def _get_nc():
    global _NC_CACHE
    if _NC_CACHE is None:
        _NC_CACHE = _build()
    return _NC_CACHE


def _run(trace=False, **inputs):
    BL = 4
    x = np.asarray(inputs["x"], np.float32)
    weight = np.asarray(inputs["weight"], np.float32)
    bias = np.asarray(inputs["bias"], np.float32)
    align = np.asarray(inputs["align"], np.float32)
    w1 = np.asarray(inputs["attn_w1"], np.float32)
    w2 = np.asarray(inputs["attn_w2"], np.float32)
    b2 = np.asarray(inputs["attn_b2"], np.float32)

    xp = np.zeros((B, I, HP, HP), dtype=ml_dtypes.bfloat16)
    xp[:, :, 1 : 1 + H, 1 : 1 + W] = x
    xe = np.ascontiguousarray(xp[:, :, :, 0::2])
    xo = np.ascontiguousarray(xp[:, :, :, 1::2])

    # host: fold align (weight-only reparam) + winograd kw-tap transform,
    # then lay out chunk-ordered:
    # wt[kh, ot, p, e, tap, ih, o] = wt_al[e, ot*128+o, ih*128+p, kh, tap]
    w_al = np.einsum("eno,eok->enk", align, weight.reshape(E, O, I * 9)).reshape(
        E, O, I, 3, 3
    )
    T = np.array(
        [[1, 0, 0], [0.5, 0.5, 0.5], [0.5, -0.5, 0.5], [0, 0, 1]], np.float32
    )
    wt = np.einsum("tk,enihk->eniht", T, w_al)  # [E, O, I, KH, TAP]
    wt = wt.reshape(E, 2, 128, 2, 128, KH, TAP)  # [e, ot, o, ih, p, kh, tap]
    wt = np.ascontiguousarray(wt.transpose(5, 1, 4, 0, 6, 3, 2)).astype(
        ml_dtypes.bfloat16
    )

    w1T = (w1 / float(H * W)).T.reshape(2, 128, HID)
    w2Ta = np.concatenate([w2.T, b2.reshape(1, E)], axis=0)
    misc = np.zeros((128, 391), np.float32)
    misc[:, 0:HID] = w1T[0]
    misc[:, HID : 2 * HID] = w1T[1]
    misc[: HID + 1, 130:135] = w2Ta
    misc[:E, 135:391] = bias

    nc = _get_nc()
    in_maps = []
    for c in range(N_CORES):
        in_maps.append(
            {
                "xe": xe[c * BL : (c + 1) * BL],
                "xo": xo[c * BL : (c + 1) * BL],
                "wt": wt,
                "misc": misc,
            }
        )
    if trace:
        _install_ntff_hook()
    res = run_bass_kernel_spmd(
        nc, in_maps, core_ids=list(range(N_CORES)), trace=trace
    )
    out = np.concatenate([res.results[c]["out"] for c in range(N_CORES)], axis=0)
    # interleave the parity planes: [B,O,H,2,28] -> [B,O,H,56]
    out = out.transpose(0, 1, 2, 4, 3).reshape(B, O, H, W)
    return out.astype(np.float32), res


def kernel(**inputs):
    out, _ = _run(trace=False, **inputs)
    return out


def kernel_profiled(**inputs):
    out, res = _run(trace=True, **inputs)
    return out, res

